# revision 8
# baseline (speedup 1.0000x reference)
"""Trainium2 Bass kernel for nn_BakaMega (EMA / damped cumulative conv).

Math: the reference's FFT causal cross-correlation with kernel
K[s,h] = alpha_h * q_h^(S-1-s), q_h = (1-alpha_h)*sigmoid(d1_h) is exactly
the first-order linear recurrence

    z[t] = q * z[t-1] + x[t];   y[t] = alpha * z[t]

per (batch, channel). On-device per core (H sharded 8 ways):
  - DMA x[b] natural layout -> SBUF tiles [128 seq x 128 ch]
  - TensorE transpose 128x128 blocks -> PSUM  (seq onto the free dim)
  - VectorE tensor_tensor_scan (state = q*state + x) straight from PSUM
  - TensorE matmul with diag(alpha) stationary-side: transposes back to
    natural layout AND applies alpha in the same pass
  - ScalarE copies PSUM->SBUF, DMA out.
"""

import numpy as np

from concourse import bacc, bass, mybir
from concourse.tile import TileContext
from concourse.masks import make_identity
from concourse.bass_utils import run_bass_kernel_spmd

B, S, H = 4, 4096, 2048
NCORES = 8
HC = H // NCORES        # 256 channels per core
P = 128                 # partitions
JBLK = S // P           # 32 seq blocks
F32 = mybir.dt.float32

_CACHE = {}


def _build_bass(reps=1):
    nc = bacc.Bacc("TRN2", target_bir_lowering=False)
    x_d = nc.dram_tensor("x", [B, S, HC], F32, kind="ExternalInput")
    aux_d = nc.dram_tensor("aux", [HC, 2], F32, kind="ExternalInput")
    y_d = nc.dram_tensor("y", [B, S, HC], F32, kind="ExternalOutput")

    with TileContext(nc) as tc:
        with (
            tc.tile_pool(name="consts", bufs=1) as consts,
            tc.tile_pool(name="io", bufs=3) as io_pool,
            tc.tile_pool(name="work", bufs=2) as work,
            tc.tile_pool(name="psum", bufs=2, space="PSUM") as psum,
        ):
            ident_g = consts.tile([P, P], F32)
            make_identity(nc, ident_g)

            # aux[c, 0] = q_c, aux[c, 1] = alpha_c; load channel-major so the
            # per-channel scalars land one-per-partition.
            auxt = consts.tile([P, 2, 2], F32)
            nc.sync.dma_start(auxt[:], aux_d.rearrange("(cb p) k -> p cb k", p=P))

            # Funnel cross-engine deps through single DVE copies so derived
            # constants only depend on DVE program order (walrus limits the
            # sync-wait slots per instruction).
            ident = consts.tile([P, P], F32)
            nc.vector.tensor_copy(ident[:], ident_g[:])
            auxv = consts.tile([P, 2, 2], F32)
            nc.vector.tensor_copy(auxv[:], auxt[:])
            ones = consts.tile([P, S], F32)
            nc.vector.memset(ones[:], 1.0)

            # qb[cb]: q broadcast along the free dim for the scan's data0.
            qb = []
            adiag = []
            for cb in range(2):
                t = consts.tile([P, S], F32, tag=f"qb{cb}")
                nc.vector.tensor_scalar_mul(t[:], ones[:], auxv[:, cb, 0:1])
                qb.append(t)
                d = consts.tile([P, P], F32, tag=f"adiag{cb}")
                nc.vector.tensor_scalar_mul(d[:], ident[:], auxv[:, cb, 1:2])
                adiag.append(d)

            for rep in range(reps):
                for b in range(B):
                    for cb in range(2):
                        # x[b, 128j+p, 128cb+c] -> L[p, j*128+c]
                        L = io_pool.tile([P, S], F32, tag="L")
                        nc.sync.dma_start(
                            L[:].rearrange("p (j c) -> p j c", c=P),
                            x_d[b].rearrange("(j p) c -> p j c", p=P)[
                                :, :, cb * P : (cb + 1) * P
                            ],
                        )

                        Y = work.tile([P, S], F32, tag="Y")
                        GW = 8 * P  # 1024 free elems per psum group (2 banks)
                        for g in range(4):
                            pin = psum.tile([P, GW], F32, tag="pin")
                            for jj in range(8):
                                j = g * 8 + jj
                                nc.tensor.transpose(
                                    pin[:, jj * P : (jj + 1) * P],
                                    L[:, j * P : (j + 1) * P],
                                    ident[:],
                                )
                            init = 0.0 if g == 0 else Y[:, g * GW - 1 : g * GW]
                            nc.vector.tensor_tensor_scan(
                                Y[:, g * GW : (g + 1) * GW],
                                qb[cb][:, 0:GW],
                                pin[:],
                                init,
                                mybir.AluOpType.mult,
                                mybir.AluOpType.add,
                            )

                        O = io_pool.tile([P, S], F32, tag="O")
                        for g in range(4):
                            pout = psum.tile([P, GW], F32, tag="pout")
                            for jj in range(8):
                                j = g * 8 + jj
                                # out[s, c] = sum_k Y[k, 128j+s] * adiag[k, c]
                                #           = alpha_c * Y[c, 128j+s]
                                nc.tensor.matmul(
                                    pout[:, jj * P : (jj + 1) * P],
                                    Y[:, j * P : (j + 1) * P],
                                    adiag[cb][:],
                                )
                            nc.scalar.activation(
                                O[:, g * GW : (g + 1) * GW],
                                pout[:],
                                mybir.ActivationFunctionType.Copy,
                            )

                        nc.sync.dma_start(
                            y_d[b].rearrange("(j p) c -> p j c", p=P)[
                                :, :, cb * P : (cb + 1) * P
                            ],
                            O[:].rearrange("p (j c) -> p j c", c=P),
                        )
    nc.finalize()
    return nc


def get_nc(reps=1):
    key = ("nc", reps)
    if key not in _CACHE:
        _CACHE[key] = _build_bass(reps)
    return _CACHE[key]


def _in_maps(x, dampeners):
    d = dampeners.astype(np.float64)
    alpha = 1.0 / (1.0 + np.exp(-d[0]))
    q = (1.0 - alpha) / (1.0 + np.exp(-d[1]))
    maps = []
    for c in range(NCORES):
        sl = slice(c * HC, (c + 1) * HC)
        aux = np.stack(
            [q[sl].astype(np.float32), alpha[sl].astype(np.float32)], axis=1
        )  # [HC, 2]
        maps.append(
            {
                "x": np.ascontiguousarray(x[:, :, sl]),
                "aux": np.ascontiguousarray(aux),
            }
        )
    return maps


def run(x, dampeners, reps=1, **spmd_kwargs):
    nc = get_nc(reps)
    res = run_bass_kernel_spmd(
        nc, _in_maps(x, dampeners), list(range(NCORES)), **spmd_kwargs
    )
    y = np.concatenate([r["y"] for r in res.results], axis=2)
    return y.astype(np.float32), res


def kernel(x, dampeners):
    y, _ = run(x, dampeners)
    return y
